# revision 11
# baseline (speedup 1.0000x reference)
"""DGI (Deep Graph Infomax) Trainium2 kernel — fused dual-pass design, v2.

Strategy (8 NeuronCores, one shared SPMD program):
  - Nodes sharded by destination: core c owns dst nodes [c*N/8, (c+1)*N/8).
  - Key identity: z2 = encode(x[perm]) needs xw[perm[src]] per edge, and
    xw[perm[n]] = (x[perm] @ W)[n].  So the device materializes fused rows
    xw_cat[n] = [ (x@W)[n] | (x_perm@W)[n] ]  (512 f16 = 1KB per row) and a
    single dma_gather feeds BOTH passes — edge indices, weights and one-hot
    structure are identical.  This halves the GPSIMD SWDGE descriptor
    generation (the dominant serial resource).
  - Gathers are merged across PAIRS of dst tiles (stream layout per pair:
    [lo_a | lo_b | hi_a | hi_b]) to amortize the ~2.8us fixed SWDGE call
    cost: 50 calls instead of 98.
  - Phase 1: host passes x^T and (x[perm])^T pre-transposed f16; 4 matmuls
    per 128 nodes accumulate [xw | xw_p] in one PSUM bank; loads ride the
    scalar engine's HWDGE ring, stores the sync ring.
  - Aggregation per pair: gather lo/hi, build the weighted one-hot S_Tw for
    all the pair's edge-tiles in 2 batched DVE ops (is_equal + in-place
    mult with 3D broadcast APs), one 512-wide matmul per edge-tile, bias
    added via a trailing K=1 ones x [b|b] matmul.  PReLU runs per dst tile
    in f32 off PSUM; z lands in a persistent f16 zcat and z1 column sums
    accumulate incrementally (so the AllReduce fires right after the last
    tile).
  - Tail: colsum matmul, 1KB AllReduce, sigmoid, wsum = disc_W @ summary,
    zcat *= [wsum|wsum] in place, one strided reduce -> [128, DT*2].
"""

import os

import numpy as np

_P = 128
_LO = 32768
_C = 8


def _build_streams(sidx, ed, ew, C, NS, DT):
    """Per-core gather/weight/dstl streams, pair-grouped tile structure.

    Stream tile order per dst-tile pair p: [lo_{2p} | lo_{2p+1} | hi_{2p} |
    hi_{2p+1}], so one gather per (pair, class) covers both dst tiles.
    Returns (idx_sbuf [C,128,n_et*8] i16, w_sbuf [C,128,n_et] f16,
             dl_sbuf [C,128,n_et] f16, Tm [NP,2,2] int, Off [NP,2,2] int,
             n_et)
    """
    NP = -(-DT // 2)
    core = ed // NS
    ldst = ed - core * NS
    dt = ldst // _P
    dstl = ldst % _P
    cls = (sidx >= _LO).astype(np.int64)
    pid = dt // 2
    sub = dt % 2

    gid = ((core * NP + pid) * 2 + cls) * 2 + sub
    NG = C * NP * 4
    cnt = np.bincount(gid, minlength=NG).reshape(C, NP, 2, 2)
    T = -(-cnt // _P)
    Tm = T.max(axis=0)  # shared structure across cores [NP, 2, 2]
    flat = Tm.reshape(-1)
    Off = np.concatenate([[0], np.cumsum(flat)[:-1]]).reshape(NP, 2, 2)
    n_et = int(flat.sum())

    order = np.argsort(gid, kind="stable")
    sorted_gid = gid[order]
    g_starts = np.concatenate(
        [[0], np.cumsum(np.bincount(sorted_gid, minlength=NG))[:-1]]
    )
    rank = np.arange(order.size) - g_starts[sorted_gid]
    g_sub = sorted_gid % 2
    g_cls = (sorted_gid // 2) % 2
    g_pid = (sorted_gid // 4) % NP
    pos = Off[g_pid, g_cls, g_sub] * _P + rank
    core_s = sorted_gid // (NP * 4)

    L = n_et * _P
    idx16 = np.zeros((C, L), np.int16)
    wv = np.zeros((C, L), np.float16)
    dl = np.full((C, L), -1.0, np.float16)
    sidx_s = sidx[order]
    idx16[core_s, pos] = (sidx_s - g_cls * _LO).astype(np.int16)
    wv[core_s, pos] = ew[order].astype(np.float16)
    dl[core_s, pos] = dstl[order].astype(np.float16)

    idx_w = idx16.reshape(C, L // 16, 16).transpose(0, 2, 1)
    idx_sbuf = np.ascontiguousarray(np.tile(idx_w, (1, 8, 1)))
    w_sbuf = np.ascontiguousarray(wv.reshape(C, n_et, _P).transpose(0, 2, 1))
    dl_sbuf = np.ascontiguousarray(dl.reshape(C, n_et, _P).transpose(0, 2, 1))
    return idx_sbuf, w_sbuf, dl_sbuf, Tm, Off, n_et


def kernel(x, W, b, a, disc_W, edge_index, perm):
    import bass_rust
    import concourse.bacc as bacc
    import concourse.mybir as mybir
    import concourse.tile as tile
    from concourse.bass_utils import run_bass_kernel_spmd

    x = np.asarray(x)
    W = np.asarray(W)
    b = np.asarray(b, np.float32)
    a = np.asarray(a, np.float32)
    disc_W = np.asarray(disc_W, np.float32)
    ei = np.asarray(edge_index, np.int64)
    perm_np = np.asarray(perm, np.int64)

    N, F = x.shape
    H = W.shape[1]
    H2 = 2 * H
    C = _C
    NS = N // C
    DT = -(-NS // _P)
    NP = -(-DT // 2)
    LAST = NS - (DT - 1) * _P  # valid rows of the last dst tile
    f16 = mybir.dt.float16
    f32 = mybir.dt.float32

    # ---- host preprocessing -------------------------------------------
    src = ei[0]
    dst = ei[1]
    deg = (np.bincount(dst, minlength=N) + 1.0).astype(np.float32)
    dinv = (1.0 / np.sqrt(deg)).astype(np.float32)
    loops = np.arange(N, dtype=np.int64)
    es = np.concatenate([src, loops])
    ed = np.concatenate([dst, loops])
    ew = dinv[es] * dinv[ed]

    i1, w1, d1, Tm, Off, n_et = _build_streams(es, ed, ew, C, NS, DT)
    maxTL = int((Tm[:, 0, 0] + Tm[:, 0, 1]).max())  # lo tiles per pair
    maxTH = int((Tm[:, 1, 0] + Tm[:, 1, 1]).max())  # hi tiles per pair
    maxTT = int(Tm.sum(axis=(1, 2)).max())          # all tiles per pair

    xT_f16 = np.ascontiguousarray(x.astype(np.float16).T)            # [F, N]
    xpT_f16 = np.ascontiguousarray(x[perm_np].astype(np.float16).T)  # [F, N]
    W_f16 = np.ascontiguousarray(W.astype(np.float16))
    b2 = np.ascontiguousarray(np.concatenate([b, b]).astype(np.float32))
    dwT = np.ascontiguousarray(disc_W.T.astype(np.float32))
    iota_np = np.tile(np.arange(_P, dtype=np.float16)[None, :], (_P, 1))
    rowmask_np = (np.arange(_P) < LAST).astype(np.float32)[:, None]

    # ---- device program -----------------------------------------------
    nc = bacc.Bacc("TRN2", target_bir_lowering=False, debug=False, num_devices=C)

    t_xT = nc.dram_tensor("xT16", [F, N], f16, kind="ExternalInput")
    t_xpT = nc.dram_tensor("xpT16", [F, N], f16, kind="ExternalInput")
    t_W = nc.dram_tensor("w16", [F, H], f16, kind="ExternalInput")
    t_b2 = nc.dram_tensor("b2vec", [H2], f32, kind="ExternalInput")
    t_a = nc.dram_tensor("avec", [1], f32, kind="ExternalInput")
    t_dwT = nc.dram_tensor("dwT", [H, H], f32, kind="ExternalInput")
    t_iota = nc.dram_tensor("iota", [_P, _P], f16, kind="ExternalInput")
    t_ident = nc.dram_tensor("ident_in", [_P, _P], f32, kind="ExternalInput")
    t_rowmask = nc.dram_tensor("rowmask", [_P, 1], f32, kind="ExternalInput")
    t_i1 = nc.dram_tensor("idx1", [_P, n_et * 8], mybir.dt.int16, kind="ExternalInput")
    t_w1 = nc.dram_tensor("wgt1", [_P, n_et], f16, kind="ExternalInput")
    t_d1 = nc.dram_tensor("dstl1", [_P, n_et], f16, kind="ExternalInput")

    t_out = nc.dram_tensor("pn_out", [_P, DT * 2], f32, kind="ExternalOutput")

    t_xw_lo = nc.dram_tensor("xw_lo", [_LO, H2], f16)
    t_xw_hi = nc.dram_tensor("xw_hi", [N - _LO, H2], f16)
    t_ar_in = nc.dram_tensor("ar_in", [H], f32)
    t_ar_out = nc.dram_tensor("ar_out", [H], f32, addr_space="Shared")

    CHUNK = 512  # phase-1 node columns per load

    with tile.TileContext(nc) as tc:
        import contextlib

        ctx = contextlib.ExitStack()
        consts = ctx.enter_context(tc.tile_pool(name="consts", bufs=1))
        ph1 = ctx.enter_context(tc.tile_pool(name="ph1", bufs=2))
        ph1o = ctx.enter_context(tc.tile_pool(name="ph1o", bufs=2))
        ph1ps = ctx.enter_context(tc.tile_pool(name="ph1ps", bufs=2, space="PSUM"))
        glo = ctx.enter_context(tc.tile_pool(name="glo", bufs=2))
        ghi = ctx.enter_context(tc.tile_pool(name="ghi", bufs=2))
        stp = ctx.enter_context(tc.tile_pool(name="stp", bufs=2))
        aggps = ctx.enter_context(tc.tile_pool(name="aggps", bufs=4, space="PSUM"))
        misc = ctx.enter_context(tc.tile_pool(name="misc", bufs=2))
        miscps = ctx.enter_context(tc.tile_pool(name="miscps", bufs=1, space="PSUM"))

        # ---- constants ----
        W0 = consts.tile([_P, H], f16, tag="W0")
        W1 = consts.tile([_P, H], f16, tag="W1")
        nc.scalar.dma_start(W0[:], t_W[0:_P, :])
        nc.scalar.dma_start(W1[:], t_W[_P : 2 * _P, :])
        iota_t = consts.tile([_P, _P], f16, tag="iota")
        nc.scalar.dma_start(iota_t[:], t_iota[:])
        b2_sb = consts.tile([1, H2], f32, tag="b2_sb")
        nc.scalar.dma_start(b2_sb[:], t_b2[None, :])
        a_sb = consts.tile([1, 1], f32, tag="a_sb")
        nc.scalar.dma_start(a_sb[:], t_a[None, :])
        dwT0 = consts.tile([_P, H], f32, tag="dwT0")
        dwT1 = consts.tile([_P, H], f32, tag="dwT1")
        nc.scalar.dma_start(dwT0[:], t_dwT[0:_P, :])
        nc.scalar.dma_start(dwT1[:], t_dwT[_P : 2 * _P, :])
        rowmask_sb = consts.tile([_P, 1], f32, tag="rowmask")
        nc.scalar.dma_start(rowmask_sb[:], t_rowmask[:, :])
        ones_row = consts.tile([1, _P], f32, tag="ones_row")
        nc.vector.memset(ones_row[:], 1.0)
        ones_col = consts.tile([_P, 1], f32, tag="ones_col")
        nc.vector.memset(ones_col[:], 1.0)

        ab_ps = miscps.tile([_P, 1], f32, tag="mps")
        nc.tensor.matmul(ab_ps[:], ones_row[:], a_sb[:], start=True, stop=True)
        a_bc = consts.tile([_P, 1], f32, tag="a_bc")
        nc.vector.tensor_copy(a_bc[:], ab_ps[:])

        # ---- stream loads ----
        i1_sb = consts.tile([_P, n_et * 8], mybir.dt.int16, tag="i1")
        w1_sb = consts.tile([_P, n_et], f16, tag="w1")
        d1_sb = consts.tile([_P, n_et], f16, tag="d1")
        nc.scalar.dma_start(i1_sb[:], t_i1[:])
        nc.scalar.dma_start(w1_sb[:], t_w1[:])
        nc.scalar.dma_start(d1_sb[:], t_d1[:])

        # ---- phase 1: xw_cat = [x@W | x_p@W], lo rows then hi rows ----
        def phase1_range(r_start, r_end, t_dst):
            for r0 in range(r_start, r_end, CHUNK):
                cols = min(CHUNK, r_end - r0)
                xT0 = ph1.tile([_P, CHUNK], f16, tag="xT0")
                xT1 = ph1.tile([_P, CHUNK], f16, tag="xT1")
                xp0 = ph1.tile([_P, CHUNK], f16, tag="xp0")
                xp1 = ph1.tile([_P, CHUNK], f16, tag="xp1")
                nc.scalar.dma_start(xT0[:, :cols], t_xT[0:_P, r0 : r0 + cols])
                nc.scalar.dma_start(xT1[:, :cols], t_xT[_P : 2 * _P, r0 : r0 + cols])
                nc.scalar.dma_start(xp0[:, :cols], t_xpT[0:_P, r0 : r0 + cols])
                nc.scalar.dma_start(xp1[:, :cols], t_xpT[_P : 2 * _P, r0 : r0 + cols])
                for o in range(0, cols, _P):
                    m = min(_P, cols - o)
                    ps = ph1ps.tile([_P, H2], f32, tag="ph1ps")
                    nc.tensor.matmul(
                        ps[:m, 0:H], xT0[:, o : o + m], W0[:], start=True, stop=False
                    )
                    nc.tensor.matmul(
                        ps[:m, 0:H], xT1[:, o : o + m], W1[:], start=False, stop=True
                    )
                    nc.tensor.matmul(
                        ps[:m, H:H2], xp0[:, o : o + m], W0[:], start=True, stop=False
                    )
                    nc.tensor.matmul(
                        ps[:m, H:H2], xp1[:, o : o + m], W1[:], start=False, stop=True
                    )
                    xw_sb = ph1o.tile([_P, H2], f16, tag="xw_sb")
                    nc.any.tensor_copy(xw_sb[:m, :], ps[:m, :])
                    nc.sync.dma_start(
                        t_dst[r0 + o - r_start : r0 + o - r_start + m, :], xw_sb[:m, :]
                    )

        phase1_range(0, _LO, t_xw_lo)
        phase1_range(_LO, N, t_xw_hi)

        # ---- aggregation: fused sweep over dst-tile pairs ----
        zcat = consts.tile([_P, DT, H2], f16, tag="zcat")
        cacc = consts.tile([_P, H], f32, tag="cacc")
        nc.vector.memset(cacc[:], 0.0)

        for p in range(NP):
            T_la, T_lb = int(Tm[p, 0, 0]), int(Tm[p, 0, 1])
            T_ha, T_hb = int(Tm[p, 1, 0]), int(Tm[p, 1, 1])
            TL = T_la + T_lb
            TH = T_ha + T_hb
            TT = TL + TH
            o0 = int(Off[p, 0, 0])  # pair stream base; layout lo_a,lo_b,hi_a,hi_b
            gl = gh = None
            if TL:
                gl = glo.tile([_P, maxTL, H2], f16, tag="gl")
                nc.gpsimd.dma_gather(
                    gl[:, :TL, :],
                    t_xw_lo[:, :],
                    i1_sb[:, 8 * o0 : 8 * (o0 + TL)],
                    TL * _P,
                    TL * _P,
                    H2,
                    single_packet=(TL * _P <= 1024),
                )
            if TH:
                gh = ghi.tile([_P, maxTH, H2], f16, tag="gh")
                nc.gpsimd.dma_gather(
                    gh[:, :TH, :],
                    t_xw_hi[:, :],
                    i1_sb[:, 8 * (o0 + TL) : 8 * (o0 + TT)],
                    TH * _P,
                    TH * _P,
                    H2,
                    single_packet=(TH * _P <= 1024),
                )
            # batched weighted one-hot for all the pair's edge-tiles
            stw = stp.tile([_P, maxTT, _P], f16, tag="stw")
            nc.vector.tensor_tensor(
                stw[:, :TT, :],
                d1_sb[:, o0 : o0 + TT, None].to_broadcast([_P, TT, _P]),
                iota_t[:, None, :].to_broadcast([_P, TT, _P]),
                mybir.AluOpType.is_equal,
            )
            nc.vector.tensor_tensor(
                stw[:, :TT, :],
                stw[:, :TT, :],
                w1_sb[:, o0 : o0 + TT, None].to_broadcast([_P, TT, _P]),
                mybir.AluOpType.mult,
            )
            for s in range(2):
                dti = 2 * p + s
                if dti >= DT:
                    break
                tl0 = s * T_la          # first lo tile (gl index) of this sub
                tln = T_la if s == 0 else T_lb
                th0 = s * T_ha
                thn = T_ha if s == 0 else T_hb
                ps = aggps.tile([_P, H2], f32, tag="aggps")
                n_mm = tln + thn
                k = 0
                for j in range(tln):
                    nc.tensor.matmul(
                        ps[:],
                        stw[:, tl0 + j, :],
                        gl[:, tl0 + j, :],
                        start=(k == 0),
                        stop=False,
                    )
                    k += 1
                for j in range(thn):
                    nc.tensor.matmul(
                        ps[:],
                        stw[:, TL + th0 + j, :],
                        gh[:, th0 + j, :],
                        start=(k == 0),
                        stop=False,
                    )
                    k += 1
                # bias via K=1 ones x [b|b] matmul (also closes the group)
                nc.tensor.matmul(ps[:], ones_row[:], b2_sb[:], start=(k == 0), stop=True)
                # PReLU in f32 off PSUM
                t1 = misc.tile([_P, H2], f32, tag="t1")
                nc.vector.tensor_scalar(
                    t1[:], ps[:], 0.0, a_bc[:, 0:1],
                    mybir.AluOpType.min, mybir.AluOpType.mult,
                )
                t2 = misc.tile([_P, H2], f32, tag="t2")
                nc.vector.tensor_scalar(t2[:], ps[:], 0.0, None, mybir.AluOpType.max)
                nc.vector.tensor_tensor(t1[:], t1[:], t2[:], mybir.AluOpType.add)
                if dti == DT - 1 and LAST < _P:
                    nc.vector.tensor_scalar(
                        t1[:], t1[:], rowmask_sb[:, 0:1], None, mybir.AluOpType.mult
                    )
                nc.any.tensor_copy(zcat[:, dti, :], t1[:])
                nc.vector.tensor_tensor(
                    cacc[:], cacc[:], t1[:, 0:H], mybir.AluOpType.add
                )

        # ---- summary: column sums of z1 over all nodes ----
        cs_ps = miscps.tile([1, H], f32, tag="mps")
        nc.tensor.matmul(cs_ps[:], ones_col[:], cacc[:], start=True, stop=True)
        cs_sb = misc.tile([1, H], f32, tag="cs_sb")
        nc.vector.tensor_copy(cs_sb[:], cs_ps[:])
        nc.sync.dma_start(t_ar_in[None, :], cs_sb[:])
        nc.gpsimd.collective_compute(
            "AllReduce",
            mybir.AluOpType.add,
            replica_groups=[list(range(C))],
            ins=[t_ar_in[:]],
            outs=[t_ar_out[:]],
        )
        sums_sb = misc.tile([1, H], f32, tag="sums_sb")
        nc.sync.dma_start(sums_sb[:], t_ar_out[None, :])
        summ_sb = misc.tile([1, H], f32, tag="summ_sb")
        nc.scalar.activation(
            summ_sb[:], sums_sb[:], mybir.ActivationFunctionType.Sigmoid,
            scale=1.0 / N,
        )

        # ---- wsum = disc_W @ summary ----
        ident = consts.tile([_P, _P], f32, tag="ident")
        nc.scalar.dma_start(ident[:], t_ident[:])
        sT = misc.tile([_P, 2], f32, tag="sT")
        for c_i in range(2):
            tp = miscps.tile([_P, _P], f32, tag="mps")
            nc.tensor.transpose(
                tp[:, 0:1],
                summ_sb[0:1, c_i * _P : (c_i + 1) * _P],
                ident[0:1, 0:1],
            )
            nc.vector.tensor_copy(sT[:, c_i : c_i + 1], tp[:, 0:1])
        ws_ps = miscps.tile([1, H], f32, tag="mps")
        nc.tensor.matmul(ws_ps[:], sT[:, 0:1], dwT0[:], start=True, stop=False)
        nc.tensor.matmul(ws_ps[:], sT[:, 1:2], dwT1[:], start=False, stop=True)
        ws2_sb = misc.tile([1, H2], f32, tag="ws2_sb")
        nc.vector.tensor_copy(ws2_sb[:, 0:H], ws_ps[:])
        nc.vector.tensor_copy(ws2_sb[:, H:H2], ws_ps[:])
        wb_ps = miscps.tile([_P, H2], f32, tag="mps")
        nc.tensor.matmul(wb_ps[:], ones_row[:], ws2_sb[:], start=True, stop=True)
        wsum_bc = consts.tile([_P, H2], f16, tag="wsum_bc")
        nc.vector.tensor_copy(wsum_bc[:], wb_ps[:])

        # ---- pos/neg dots: zcat *= [wsum|wsum]; reduce 256-chunks ----
        nc.vector.tensor_tensor(
            zcat[:, :, :],
            zcat[:, :, :],
            wsum_bc[:, None, :].to_broadcast([_P, DT, H2]),
            mybir.AluOpType.mult,
        )
        out_acc = misc.tile([_P, DT * 2], f32, tag="out_acc")
        nc.vector.reduce_sum(
            out_acc[:],
            zcat[:, :, :].rearrange("p d (t h) -> p (d t) h", t=2, h=H),
            bass_rust.AxisListType.X,
        )
        nc.sync.dma_start(t_out[:], out_acc[:])
        ctx.close()

    nc.compile()

    in_maps = []
    for c in range(C):
        in_maps.append(
            {
                "xT16": xT_f16,
                "xpT16": xpT_f16,
                "w16": W_f16,
                "b2vec": b2,
                "avec": a,
                "dwT": dwT,
                "iota": iota_np,
                "ident_in": np.eye(_P, dtype=np.float32),
                "rowmask": rowmask_np,
                "idx1": i1[c],
                "wgt1": w1[c],
                "dstl1": d1[c],
            }
        )

    if os.environ.get("KERNEL_SIM", "0") == "1":
        from concourse import bass_interp

        sim = bass_interp.MultiCoreSim(nc, C)
        for c in range(C):
            for k, v in in_maps[c].items():
                sim.cores[c].tensor(k)[:] = v
        sim.simulate()
        results = [
            {"pn_out": np.array(sim.cores[c].tensor("pn_out"))} for c in range(C)
        ]
    else:
        trace = os.environ.get("KERNEL_TRACE", "0") == "1"
        kw = {}
        if trace:
            kw["trace"] = True
        res = run_bass_kernel_spmd(nc, in_maps, core_ids=list(range(C)), **kw)
        kernel.last_result = res
        results = res.results

    pos = np.zeros(N, np.float32)
    neg = np.zeros(N, np.float32)
    for c in range(C):
        arr = results[c]["pn_out"].reshape(_P, DT, 2)
        pos[c * NS : (c + 1) * NS] = arr[:, :, 0].T.reshape(-1)[:NS]
        neg[c * NS : (c + 1) * NS] = arr[:, :, 1].T.reshape(-1)[:NS]
    return pos, neg


# revision 16
# speedup vs baseline: 1.2039x; 1.2039x over previous
"""DGI (Deep Graph Infomax) Trainium2 kernel — fused dual-pass design, v2.

Strategy (8 NeuronCores, one shared SPMD program):
  - Nodes sharded by destination: core c owns dst nodes [c*N/8, (c+1)*N/8).
  - Key identity: z2 = encode(x[perm]) needs xw[perm[src]] per edge, and
    xw[perm[n]] = (x[perm] @ W)[n].  So the device materializes fused rows
    xw_cat[n] = [ (x@W)[n] | (x_perm@W)[n] ]  (512 f16 = 1KB per row) and a
    single dma_gather feeds BOTH passes — edge indices, weights and one-hot
    structure are identical.  This halves the GPSIMD SWDGE descriptor
    generation (the dominant serial resource).
  - Gathers are merged across PAIRS of dst tiles (stream layout per pair:
    [lo_a | lo_b | hi_a | hi_b]) to amortize the ~2.8us fixed SWDGE call
    cost: 50 calls instead of 98.
  - Phase 1: host passes x^T and (x[perm])^T pre-transposed f16; 4 matmuls
    per 128 nodes accumulate [xw | xw_p] in one PSUM bank; loads ride the
    scalar engine's HWDGE ring, stores the sync ring.
  - Aggregation per pair: gather lo/hi, build the weighted one-hot S_Tw for
    all the pair's edge-tiles in 2 batched DVE ops (is_equal + in-place
    mult with 3D broadcast APs), one 512-wide matmul per edge-tile, bias
    added via a trailing K=1 ones x [b|b] matmul.  PReLU runs per dst tile
    in f32 off PSUM; z lands in a persistent f16 zcat and z1 column sums
    accumulate incrementally (so the AllReduce fires right after the last
    tile).
  - Tail: colsum matmul, 1KB AllReduce, sigmoid, wsum = disc_W @ summary,
    zcat *= [wsum|wsum] in place, one strided reduce -> [128, DT*2].
"""

import os

import numpy as np

_P = 128
_LO = 32768
_C = 8


def _build_streams(sidx, ed, ew, C, NS, DT):
    """Per-core gather/weight/dstl streams, pair-grouped tile structure.

    Stream tile order per dst-tile pair p: [lo_{2p} | lo_{2p+1} | hi_{2p} |
    hi_{2p+1}], so one gather per (pair, class) covers both dst tiles.
    Returns (idx_sbuf [C,128,n_et*8] i16, w_sbuf [C,128,n_et] f16,
             dl_sbuf [C,128,n_et] f16, Tm [NP,2,2] int, Off [NP,2,2] int,
             n_et)
    """
    NP = -(-DT // 2)
    core = ed // NS
    ldst = ed - core * NS
    dt = ldst // _P
    dstl = ldst % _P
    cls = (sidx >= _LO).astype(np.int64)
    pid = dt // 2
    sub = dt % 2

    gid = ((core * NP + pid) * 2 + cls) * 2 + sub
    NG = C * NP * 4
    cnt = np.bincount(gid, minlength=NG).reshape(C, NP, 2, 2)
    T = -(-cnt // _P)
    Tm = T.max(axis=0)  # shared structure across cores [NP, 2, 2]
    flat = Tm.reshape(-1)
    Off = np.concatenate([[0], np.cumsum(flat)[:-1]]).reshape(NP, 2, 2)
    n_et = int(flat.sum())

    order = np.argsort(gid, kind="stable")
    sorted_gid = gid[order]
    g_starts = np.concatenate(
        [[0], np.cumsum(np.bincount(sorted_gid, minlength=NG))[:-1]]
    )
    rank = np.arange(order.size) - g_starts[sorted_gid]
    g_sub = sorted_gid % 2
    g_cls = (sorted_gid // 2) % 2
    g_pid = (sorted_gid // 4) % NP
    pos = Off[g_pid, g_cls, g_sub] * _P + rank
    core_s = sorted_gid // (NP * 4)

    L = n_et * _P
    idx16 = np.zeros((C, L), np.int16)
    wv = np.zeros((C, L), np.float16)
    dl = np.full((C, L), -1.0, np.float16)
    sidx_s = sidx[order]
    idx16[core_s, pos] = (sidx_s - g_cls * _LO).astype(np.int16)
    wv[core_s, pos] = ew[order].astype(np.float16)
    dl[core_s, pos] = dstl[order].astype(np.float16)

    idx_w = idx16.reshape(C, L // 16, 16).transpose(0, 2, 1)
    idx_sbuf = np.ascontiguousarray(np.tile(idx_w, (1, 8, 1)))
    w_sbuf = np.ascontiguousarray(wv.reshape(C, n_et, _P).transpose(0, 2, 1))
    dl_sbuf = np.ascontiguousarray(dl.reshape(C, n_et, _P).transpose(0, 2, 1))
    return idx_sbuf, w_sbuf, dl_sbuf, Tm, Off, n_et


def kernel(x, W, b, a, disc_W, edge_index, perm):
    import bass_rust
    import concourse.bacc as bacc
    import concourse.mybir as mybir
    import concourse.tile as tile
    from concourse.bass_utils import run_bass_kernel_spmd

    x = np.asarray(x)
    W = np.asarray(W)
    b = np.asarray(b, np.float32)
    a = np.asarray(a, np.float32)
    disc_W = np.asarray(disc_W, np.float32)
    ei = np.asarray(edge_index, np.int64)
    perm_np = np.asarray(perm, np.int64)

    N, F = x.shape
    H = W.shape[1]
    H2 = 2 * H
    C = _C
    NS = N // C
    DT = -(-NS // _P)
    NP = -(-DT // 2)
    LAST = NS - (DT - 1) * _P  # valid rows of the last dst tile
    f16 = mybir.dt.float16
    f32 = mybir.dt.float32

    # ---- host preprocessing -------------------------------------------
    src = ei[0]
    dst = ei[1]
    deg = (np.bincount(dst, minlength=N) + 1.0).astype(np.float32)
    dinv = (1.0 / np.sqrt(deg)).astype(np.float32)
    loops = np.arange(N, dtype=np.int64)
    es = np.concatenate([src, loops])
    ed = np.concatenate([dst, loops])
    ew = dinv[es] * dinv[ed]

    i1, w1, d1, Tm, Off, n_et = _build_streams(es, ed, ew, C, NS, DT)
    maxTL = int((Tm[:, 0, 0] + Tm[:, 0, 1]).max())  # lo tiles per pair
    maxTH = int((Tm[:, 1, 0] + Tm[:, 1, 1]).max())  # hi tiles per pair
    maxTT = int(Tm.sum(axis=(1, 2)).max())          # all tiles per pair

    xT_f16 = np.ascontiguousarray(x.astype(np.float16).T)            # [F, N]
    xpT_f16 = np.ascontiguousarray(x[perm_np].astype(np.float16).T)  # [F, N]
    W_f16 = np.ascontiguousarray(W.astype(np.float16))
    b2 = np.ascontiguousarray(np.concatenate([b, b]).astype(np.float32))
    dwT = np.ascontiguousarray(disc_W.T.astype(np.float32))
    iota_np = np.tile(np.arange(_P, dtype=np.float16)[None, :], (_P, 1))
    rowmask_np = (np.arange(_P) < LAST).astype(np.float32)[:, None]

    # ---- device program -----------------------------------------------
    nc = bacc.Bacc("TRN2", target_bir_lowering=False, debug=False, num_devices=C)

    t_xT = nc.dram_tensor("xT16", [F, N], f16, kind="ExternalInput")
    t_xpT = nc.dram_tensor("xpT16", [F, N], f16, kind="ExternalInput")
    t_W = nc.dram_tensor("w16", [F, H], f16, kind="ExternalInput")
    t_b2 = nc.dram_tensor("b2vec", [H2], f32, kind="ExternalInput")
    t_a = nc.dram_tensor("avec", [1], f32, kind="ExternalInput")
    t_dwT = nc.dram_tensor("dwT", [H, H], f32, kind="ExternalInput")
    t_iota = nc.dram_tensor("iota", [_P, _P], f16, kind="ExternalInput")
    t_ident = nc.dram_tensor("ident_in", [_P, _P], f32, kind="ExternalInput")
    t_rowmask = nc.dram_tensor("rowmask", [_P, 1], f32, kind="ExternalInput")
    t_i1 = nc.dram_tensor("idx1", [_P, n_et * 8], mybir.dt.int16, kind="ExternalInput")
    t_w1 = nc.dram_tensor("wgt1", [_P, n_et], f16, kind="ExternalInput")
    t_d1 = nc.dram_tensor("dstl1", [_P, n_et], f16, kind="ExternalInput")

    t_out = nc.dram_tensor("pn_out", [_P, DT * 2], f32, kind="ExternalOutput")

    t_xw_lo = nc.dram_tensor("xw_lo", [_LO, H2], f16)
    t_xw_hi = nc.dram_tensor("xw_hi", [N - _LO, H2], f16)
    t_ar_in = nc.dram_tensor("ar_in", [H], f32)
    t_ar_out = nc.dram_tensor("ar_out", [H], f32, addr_space="Shared")

    CHUNK = 512  # phase-1 node columns per load

    with tile.TileContext(nc) as tc:
        import contextlib

        ctx = contextlib.ExitStack()
        consts = ctx.enter_context(tc.tile_pool(name="consts", bufs=1))
        ph1 = ctx.enter_context(tc.tile_pool(name="ph1", bufs=2))
        ph1o = ctx.enter_context(tc.tile_pool(name="ph1o", bufs=2))
        ph1ps = ctx.enter_context(tc.tile_pool(name="ph1ps", bufs=2, space="PSUM"))
        glo = ctx.enter_context(tc.tile_pool(name="glo", bufs=2))
        ghi = ctx.enter_context(tc.tile_pool(name="ghi", bufs=2))
        stp = ctx.enter_context(tc.tile_pool(name="stp", bufs=2))
        aggps = ctx.enter_context(tc.tile_pool(name="aggps", bufs=4, space="PSUM"))
        misc = ctx.enter_context(tc.tile_pool(name="misc", bufs=1))
        miscps = ctx.enter_context(tc.tile_pool(name="miscps", bufs=1, space="PSUM"))

        # ---- constants ----
        W0 = consts.tile([_P, H], f16, tag="W0")
        W1 = consts.tile([_P, H], f16, tag="W1")
        nc.scalar.dma_start(W0[:], t_W[0:_P, :])
        nc.scalar.dma_start(W1[:], t_W[_P : 2 * _P, :])
        iota_t = consts.tile([_P, _P], f16, tag="iota")
        nc.scalar.dma_start(iota_t[:], t_iota[:])
        b2_sb = consts.tile([1, H2], f32, tag="b2_sb")
        nc.scalar.dma_start(b2_sb[:], t_b2[None, :])
        a_sb = consts.tile([1, 1], f32, tag="a_sb")
        nc.scalar.dma_start(a_sb[:], t_a[None, :])
        dwT0 = consts.tile([_P, H], f32, tag="dwT0")
        dwT1 = consts.tile([_P, H], f32, tag="dwT1")
        nc.scalar.dma_start(dwT0[:], t_dwT[0:_P, :])
        nc.scalar.dma_start(dwT1[:], t_dwT[_P : 2 * _P, :])
        rowmask_sb = consts.tile([_P, 1], f32, tag="rowmask")
        nc.scalar.dma_start(rowmask_sb[:], t_rowmask[:, :])
        ones_row = consts.tile([1, _P], f32, tag="ones_row")
        nc.vector.memset(ones_row[:], 1.0)
        ones_col = consts.tile([_P, 1], f32, tag="ones_col")
        nc.vector.memset(ones_col[:], 1.0)

        ab_ps = miscps.tile([_P, 1], f32, tag="mps")
        nc.tensor.matmul(ab_ps[:], ones_row[:], a_sb[:], start=True, stop=True)
        a_bc = consts.tile([_P, 1], f32, tag="a_bc")
        nc.vector.tensor_copy(a_bc[:], ab_ps[:])

        # ---- stream loads ----
        i1_sb = consts.tile([_P, n_et * 8], mybir.dt.int16, tag="i1")
        w1_sb = consts.tile([_P, n_et], f16, tag="w1")
        d1_sb = consts.tile([_P, n_et], f16, tag="d1")
        nc.scalar.dma_start(i1_sb[:], t_i1[:])
        nc.scalar.dma_start(w1_sb[:], t_w1[:])
        nc.scalar.dma_start(d1_sb[:], t_d1[:])

        # ---- phase 1: xw_cat = [x@W | x_p@W], hi rows first ----
        def phase1_range(r_start, r_end, t_dst):
            for r0 in range(r_start, r_end, CHUNK):
                cols = min(CHUNK, r_end - r0)
                xT0 = ph1.tile([_P, CHUNK], f16, tag="xT0")
                xT1 = ph1.tile([_P, CHUNK], f16, tag="xT1")
                xp0 = ph1.tile([_P, CHUNK], f16, tag="xp0")
                xp1 = ph1.tile([_P, CHUNK], f16, tag="xp1")
                nc.sync.dma_start(xT0[:, :cols], t_xT[0:_P, r0 : r0 + cols])
                nc.sync.dma_start(xT1[:, :cols], t_xT[_P : 2 * _P, r0 : r0 + cols])
                nc.sync.dma_start(xp0[:, :cols], t_xpT[0:_P, r0 : r0 + cols])
                nc.sync.dma_start(xp1[:, :cols], t_xpT[_P : 2 * _P, r0 : r0 + cols])
                # store in 4-group (512-row) batches to cut DMA-issue count
                for s0 in range(0, cols, 4 * _P):
                    srows = min(4 * _P, cols - s0)
                    ng = -(-srows // _P)
                    stage = ph1o.tile([_P, 4, H2], f16, tag="xw_sb")
                    for gi in range(ng):
                        o = s0 + gi * _P
                        m = min(_P, cols - o)
                        ps = ph1ps.tile([_P, H2], f32, tag="ph1ps")
                        nc.tensor.matmul(
                            ps[:m, 0:H], xT0[:, o : o + m], W0[:],
                            start=True, stop=False,
                        )
                        nc.tensor.matmul(
                            ps[:m, 0:H], xT1[:, o : o + m], W1[:],
                            start=False, stop=True,
                        )
                        nc.tensor.matmul(
                            ps[:m, H:H2], xp0[:, o : o + m], W0[:],
                            start=True, stop=False,
                        )
                        nc.tensor.matmul(
                            ps[:m, H:H2], xp1[:, o : o + m], W1[:],
                            start=False, stop=True,
                        )
                        nc.any.tensor_copy(stage[:m, gi, :], ps[:m, :])
                    d0 = r0 + s0 - r_start
                    if srows % _P == 0:
                        nc.sync.dma_start(
                            t_dst[d0 : d0 + srows, :].rearrange(
                                "(g p) h -> p g h", g=ng, p=_P
                            ),
                            stage[:, :ng, :],
                        )
                    else:
                        for gi in range(ng):
                            m = min(_P, srows - gi * _P)
                            nc.sync.dma_start(
                                t_dst[d0 + gi * _P : d0 + gi * _P + m, :],
                                stage[:m, gi, :],
                            )

        phase1_range(_LO, N, t_xw_hi)
        phase1_range(0, _LO, t_xw_lo)

        # ---- aggregation: fused sweep over dst-tile pairs ----
        zcat = consts.tile([_P, DT, H2], f16, tag="zcat")
        cacc = consts.tile([_P, H], f32, tag="cacc")
        nc.vector.memset(cacc[:], 0.0)

        for p in range(NP):
            T_la, T_lb = int(Tm[p, 0, 0]), int(Tm[p, 0, 1])
            T_ha, T_hb = int(Tm[p, 1, 0]), int(Tm[p, 1, 1])
            TL = T_la + T_lb
            TH = T_ha + T_hb
            TT = TL + TH
            o0 = int(Off[p, 0, 0])  # pair stream base; layout lo_a,lo_b,hi_a,hi_b
            gl = gh = None
            if TH:  # hi rows are written first in phase 1 — gather them first
                gh = ghi.tile([_P, maxTH, H2], f16, tag="gh")
                nc.gpsimd.dma_gather(
                    gh[:, :TH, :],
                    t_xw_hi[:, :],
                    i1_sb[:, 8 * (o0 + TL) : 8 * (o0 + TT)],
                    TH * _P,
                    TH * _P,
                    H2,
                    single_packet=(TH * _P <= 1024),
                )
            if TL:
                gl = glo.tile([_P, maxTL, H2], f16, tag="gl")
                nc.gpsimd.dma_gather(
                    gl[:, :TL, :],
                    t_xw_lo[:, :],
                    i1_sb[:, 8 * o0 : 8 * (o0 + TL)],
                    TL * _P,
                    TL * _P,
                    H2,
                    single_packet=(TL * _P <= 1024),
                )
            # batched weighted one-hot for all the pair's edge-tiles
            stw = stp.tile([_P, maxTT, _P], f16, tag="stw")
            nc.vector.tensor_tensor(
                stw[:, :TT, :],
                d1_sb[:, o0 : o0 + TT, None].to_broadcast([_P, TT, _P]),
                iota_t[:, None, :].to_broadcast([_P, TT, _P]),
                mybir.AluOpType.is_equal,
            )
            nc.vector.tensor_tensor(
                stw[:, :TT, :],
                stw[:, :TT, :],
                w1_sb[:, o0 : o0 + TT, None].to_broadcast([_P, TT, _P]),
                mybir.AluOpType.mult,
            )
            for s in range(2):
                dti = 2 * p + s
                if dti >= DT:
                    break
                tl0 = s * T_la          # first lo tile (gl index) of this sub
                tln = T_la if s == 0 else T_lb
                th0 = s * T_ha
                thn = T_ha if s == 0 else T_hb
                ps = aggps.tile([_P, H2], f32, tag="aggps")
                n_mm = tln + thn
                k = 0
                for j in range(tln):
                    nc.tensor.matmul(
                        ps[:],
                        stw[:, tl0 + j, :],
                        gl[:, tl0 + j, :],
                        start=(k == 0),
                        stop=False,
                    )
                    k += 1
                for j in range(thn):
                    nc.tensor.matmul(
                        ps[:],
                        stw[:, TL + th0 + j, :],
                        gh[:, th0 + j, :],
                        start=(k == 0),
                        stop=False,
                    )
                    k += 1
                # bias via K=1 ones x [b|b] matmul (also closes the group)
                nc.tensor.matmul(ps[:], ones_row[:], b2_sb[:], start=(k == 0), stop=True)
                # PReLU in f32 off PSUM
                t1 = misc.tile([_P, H2], f32, tag="t1")
                nc.vector.tensor_scalar(
                    t1[:], ps[:], 0.0, a_bc[:, 0:1],
                    mybir.AluOpType.min, mybir.AluOpType.mult,
                )
                t2 = misc.tile([_P, H2], f32, tag="t2")
                nc.vector.tensor_scalar(t2[:], ps[:], 0.0, None, mybir.AluOpType.max)
                nc.vector.tensor_tensor(t1[:], t1[:], t2[:], mybir.AluOpType.add)
                if dti == DT - 1 and LAST < _P:
                    nc.vector.tensor_scalar(
                        t1[:], t1[:], rowmask_sb[:, 0:1], None, mybir.AluOpType.mult
                    )
                nc.any.tensor_copy(zcat[:, dti, :], t1[:])
                nc.vector.tensor_tensor(
                    cacc[:], cacc[:], t1[:, 0:H], mybir.AluOpType.add
                )

        # ---- summary: column sums of z1 over all nodes ----
        cs_ps = miscps.tile([1, H], f32, tag="mps")
        nc.tensor.matmul(cs_ps[:], ones_col[:], cacc[:], start=True, stop=True)
        cs_sb = misc.tile([1, H], f32, tag="cs_sb")
        nc.vector.tensor_copy(cs_sb[:], cs_ps[:])
        nc.sync.dma_start(t_ar_in[None, :], cs_sb[:])
        nc.gpsimd.collective_compute(
            "AllReduce",
            mybir.AluOpType.add,
            replica_groups=[list(range(C))],
            ins=[t_ar_in[:]],
            outs=[t_ar_out[:]],
        )
        sums_sb = misc.tile([1, H], f32, tag="sums_sb")
        nc.sync.dma_start(sums_sb[:], t_ar_out[None, :])
        summ_sb = misc.tile([1, H], f32, tag="summ_sb")
        nc.scalar.activation(
            summ_sb[:], sums_sb[:], mybir.ActivationFunctionType.Sigmoid,
            scale=1.0 / N,
        )

        # ---- wsum = disc_W @ summary ----
        ident = consts.tile([_P, _P], f32, tag="ident")
        nc.scalar.dma_start(ident[:], t_ident[:])
        sT = misc.tile([_P, 2], f32, tag="sT")
        for c_i in range(2):
            tp = miscps.tile([_P, _P], f32, tag="mps")
            nc.tensor.transpose(
                tp[:, 0:1],
                summ_sb[0:1, c_i * _P : (c_i + 1) * _P],
                ident[0:1, 0:1],
            )
            nc.vector.tensor_copy(sT[:, c_i : c_i + 1], tp[:, 0:1])
        ws_ps = miscps.tile([1, H], f32, tag="mps")
        nc.tensor.matmul(ws_ps[:], sT[:, 0:1], dwT0[:], start=True, stop=False)
        nc.tensor.matmul(ws_ps[:], sT[:, 1:2], dwT1[:], start=False, stop=True)
        ws2_sb = misc.tile([1, H2], f32, tag="ws2_sb")
        nc.vector.tensor_copy(ws2_sb[:, 0:H], ws_ps[:])
        nc.vector.tensor_copy(ws2_sb[:, H:H2], ws_ps[:])
        wb_ps = miscps.tile([_P, H2], f32, tag="mps")
        nc.tensor.matmul(wb_ps[:], ones_row[:], ws2_sb[:], start=True, stop=True)
        wsum_bc = consts.tile([_P, H2], f16, tag="wsum_bc")
        nc.vector.tensor_copy(wsum_bc[:], wb_ps[:])

        # ---- pos/neg dots: zcat *= [wsum|wsum]; reduce 256-chunks ----
        nc.vector.tensor_tensor(
            zcat[:, :, :],
            zcat[:, :, :],
            wsum_bc[:, None, :].to_broadcast([_P, DT, H2]),
            mybir.AluOpType.mult,
        )
        out_acc = misc.tile([_P, DT * 2], f32, tag="out_acc")
        nc.vector.reduce_sum(
            out_acc[:],
            zcat[:, :, :].rearrange("p d (t h) -> p (d t) h", t=2, h=H),
            bass_rust.AxisListType.X,
        )
        nc.sync.dma_start(t_out[:], out_acc[:])
        ctx.close()

    nc.compile()

    in_maps = []
    for c in range(C):
        in_maps.append(
            {
                "xT16": xT_f16,
                "xpT16": xpT_f16,
                "w16": W_f16,
                "b2vec": b2,
                "avec": a,
                "dwT": dwT,
                "iota": iota_np,
                "ident_in": np.eye(_P, dtype=np.float32),
                "rowmask": rowmask_np,
                "idx1": i1[c],
                "wgt1": w1[c],
                "dstl1": d1[c],
            }
        )

    if os.environ.get("KERNEL_SIM", "0") == "1":
        from concourse import bass_interp

        sim = bass_interp.MultiCoreSim(nc, C)
        for c in range(C):
            for k, v in in_maps[c].items():
                sim.cores[c].tensor(k)[:] = v
        sim.simulate()
        results = [
            {"pn_out": np.array(sim.cores[c].tensor("pn_out"))} for c in range(C)
        ]
    else:
        trace = os.environ.get("KERNEL_TRACE", "0") == "1"
        kw = {}
        if trace:
            kw["trace"] = True
        res = run_bass_kernel_spmd(nc, in_maps, core_ids=list(range(C)), **kw)
        kernel.last_result = res
        results = res.results

    pos = np.zeros(N, np.float32)
    neg = np.zeros(N, np.float32)
    for c in range(C):
        arr = results[c]["pn_out"].reshape(_P, DT, 2)
        pos[c * NS : (c + 1) * NS] = arr[:, :, 0].T.reshape(-1)[:NS]
        neg[c * NS : (c + 1) * NS] = arr[:, :, 1].T.reshape(-1)[:NS]
    return pos, neg


# revision 18
# speedup vs baseline: 1.2352x; 1.0260x over previous
"""DGI (Deep Graph Infomax) Trainium2 kernel — fused dual-pass design, v2.

Strategy (8 NeuronCores, one shared SPMD program):
  - Nodes sharded by destination: core c owns dst nodes [c*N/8, (c+1)*N/8).
  - Key identity: z2 = encode(x[perm]) needs xw[perm[src]] per edge, and
    xw[perm[n]] = (x[perm] @ W)[n].  So the device materializes fused rows
    xw_cat[n] = [ (x@W)[n] | (x_perm@W)[n] ]  (512 f16 = 1KB per row) and a
    single dma_gather feeds BOTH passes — edge indices, weights and one-hot
    structure are identical.  This halves the GPSIMD SWDGE descriptor
    generation (the dominant serial resource).
  - Gathers are merged across PAIRS of dst tiles (stream layout per pair:
    [lo_a | lo_b | hi_a | hi_b]): 50 SWDGE calls instead of 98.
  - Phase 1: host passes x^T and (x[perm])^T pre-transposed f16; 4 matmuls
    per 128 nodes accumulate [xw | xw_p] in one PSUM bank; stores are
    batched 4 groups per DMA (DMA-issue cost ~0.6us/instr was binding);
    hi rows are produced first so the per-pair hi gathers never stall.
  - Aggregation per pair: gather lo/hi, build the weighted one-hot S_Tw for
    all the pair's edge-tiles in 2 batched DVE ops (is_equal + in-place
    mult with 3D broadcast APs), one 512-wide matmul per edge-tile, bias
    added via a trailing K=1 ones x [b|b] matmul.  PReLU runs per dst tile
    in f32 off PSUM; z lands in a persistent f16 zcat and z1 column sums
    accumulate incrementally (so the AllReduce fires right after the last
    tile).
  - Tail: colsum matmul, 1KB AllReduce, sigmoid, wsum = disc_W @ summary,
    zcat *= [wsum|wsum] in place, one strided reduce -> [128, DT*2].
"""

import os

import numpy as np

_P = 128
_LO = 32768
_C = 8


def _build_streams(sidx, ed, ew, C, NS, DT):
    """Per-core gather/weight/dstl streams, pair-grouped tile structure.

    Stream tile order per dst-tile pair p: [lo_{2p} | lo_{2p+1} | hi_{2p} |
    hi_{2p+1}], so one gather per (pair, class) covers both dst tiles.
    Returns (idx_sbuf [C,128,n_et*8] i16, w_sbuf [C,128,n_et] f16,
             dl_sbuf [C,128,n_et] f16, Tm [NP,2,2] int, Off [NP,2,2] int,
             n_et)
    """
    NP = -(-DT // 2)
    core = ed // NS
    ldst = ed - core * NS
    dt = ldst // _P
    dstl = ldst % _P
    cls = (sidx >= _LO).astype(np.int64)
    pid = dt // 2
    sub = dt % 2

    gid = ((core * NP + pid) * 2 + cls) * 2 + sub
    NG = C * NP * 4
    cnt = np.bincount(gid, minlength=NG).reshape(C, NP, 2, 2)
    T = -(-cnt // _P)
    Tm = T.max(axis=0)  # shared structure across cores [NP, 2, 2]
    flat = Tm.reshape(-1)
    Off = np.concatenate([[0], np.cumsum(flat)[:-1]]).reshape(NP, 2, 2)
    n_et = int(flat.sum())

    order = np.argsort(gid, kind="stable")
    sorted_gid = gid[order]
    g_starts = np.concatenate(
        [[0], np.cumsum(np.bincount(sorted_gid, minlength=NG))[:-1]]
    )
    rank = np.arange(order.size) - g_starts[sorted_gid]
    g_sub = sorted_gid % 2
    g_cls = (sorted_gid // 2) % 2
    g_pid = (sorted_gid // 4) % NP
    pos = Off[g_pid, g_cls, g_sub] * _P + rank
    core_s = sorted_gid // (NP * 4)

    L = n_et * _P
    idx16 = np.zeros((C, L), np.int16)
    wv = np.zeros((C, L), np.float16)
    dl = np.full((C, L), -1.0, np.float16)
    sidx_s = sidx[order]
    idx16[core_s, pos] = (sidx_s - g_cls * _LO).astype(np.int16)
    wv[core_s, pos] = ew[order].astype(np.float16)
    dl[core_s, pos] = dstl[order].astype(np.float16)

    idx_w = idx16.reshape(C, L // 16, 16).transpose(0, 2, 1)
    idx_sbuf = np.ascontiguousarray(np.tile(idx_w, (1, 8, 1)))
    w_sbuf = np.ascontiguousarray(wv.reshape(C, n_et, _P).transpose(0, 2, 1))
    dl_sbuf = np.ascontiguousarray(dl.reshape(C, n_et, _P).transpose(0, 2, 1))
    return idx_sbuf, w_sbuf, dl_sbuf, Tm, Off, n_et


def kernel(x, W, b, a, disc_W, edge_index, perm):
    import bass_rust
    import concourse.bacc as bacc
    import concourse.mybir as mybir
    import concourse.tile as tile
    from concourse.bass_utils import run_bass_kernel_spmd

    x = np.asarray(x)
    W = np.asarray(W)
    b = np.asarray(b, np.float32)
    a = np.asarray(a, np.float32)
    disc_W = np.asarray(disc_W, np.float32)
    ei = np.asarray(edge_index, np.int64)
    perm_np = np.asarray(perm, np.int64)

    N, F = x.shape
    H = W.shape[1]
    H2 = 2 * H
    C = _C
    NS = N // C
    DT = -(-NS // _P)
    NP = -(-DT // 2)
    LAST = NS - (DT - 1) * _P  # valid rows of the last dst tile
    f16 = mybir.dt.float16
    f32 = mybir.dt.float32

    # ---- host preprocessing -------------------------------------------
    src = ei[0]
    dst = ei[1]
    deg = (np.bincount(dst, minlength=N) + 1.0).astype(np.float32)
    dinv = (1.0 / np.sqrt(deg)).astype(np.float32)
    loops = np.arange(N, dtype=np.int64)
    es = np.concatenate([src, loops])
    ed = np.concatenate([dst, loops])
    ew = dinv[es] * dinv[ed]

    i1, w1, d1, Tm, Off, n_et = _build_streams(es, ed, ew, C, NS, DT)
    maxTL = int((Tm[:, 0, 0] + Tm[:, 0, 1]).max())  # lo tiles per pair
    maxTH = int((Tm[:, 1, 0] + Tm[:, 1, 1]).max())  # hi tiles per pair
    maxTT = int(Tm.sum(axis=(1, 2)).max())          # all tiles per pair

    # [128, 2, N]: partition p, F-half h -> x[:, h*128+p]; one load covers both halves
    xT_f16 = np.ascontiguousarray(
        x.astype(np.float16).T.reshape(2, _P, -1).transpose(1, 0, 2))
    xpT_f16 = np.ascontiguousarray(
        x[perm_np].astype(np.float16).T.reshape(2, _P, -1).transpose(1, 0, 2))
    W_f16 = np.ascontiguousarray(W.astype(np.float16))
    b2 = np.ascontiguousarray(np.concatenate([b, b]).astype(np.float32))
    dwT = np.ascontiguousarray(disc_W.T.astype(np.float32))
    iota_np = np.tile(np.arange(_P, dtype=np.float16)[None, :], (_P, 1))
    rowmask_np = (np.arange(_P) < LAST).astype(np.float32)[:, None]

    # ---- device program -----------------------------------------------
    nc = bacc.Bacc("TRN2", target_bir_lowering=False, debug=False, num_devices=C)

    t_xT = nc.dram_tensor("xT16", [_P, 2, N], f16, kind="ExternalInput")
    t_xpT = nc.dram_tensor("xpT16", [_P, 2, N], f16, kind="ExternalInput")
    t_W = nc.dram_tensor("w16", [F, H], f16, kind="ExternalInput")
    t_b2 = nc.dram_tensor("b2vec", [H2], f32, kind="ExternalInput")
    t_a = nc.dram_tensor("avec", [1], f32, kind="ExternalInput")
    t_dwT = nc.dram_tensor("dwT", [H, H], f32, kind="ExternalInput")
    t_iota = nc.dram_tensor("iota", [_P, _P], f16, kind="ExternalInput")
    t_ident = nc.dram_tensor("ident_in", [_P, _P], f32, kind="ExternalInput")
    t_rowmask = nc.dram_tensor("rowmask", [_P, 1], f32, kind="ExternalInput")
    t_i1 = nc.dram_tensor("idx1", [_P, n_et * 8], mybir.dt.int16, kind="ExternalInput")
    t_w1 = nc.dram_tensor("wgt1", [_P, n_et], f16, kind="ExternalInput")
    t_d1 = nc.dram_tensor("dstl1", [_P, n_et], f16, kind="ExternalInput")

    t_out = nc.dram_tensor("pn_out", [_P, DT * 2], f32, kind="ExternalOutput")

    t_xw_lo = nc.dram_tensor("xw_lo", [_LO, H2], f16)
    t_xw_hi = nc.dram_tensor("xw_hi", [N - _LO, H2], f16)
    t_ar_in = nc.dram_tensor("ar_in", [H], f32)
    t_ar_out = nc.dram_tensor("ar_out", [H], f32, addr_space="Shared")

    CHUNK = 512  # phase-1 node columns per load

    with tile.TileContext(nc) as tc:
        import contextlib

        ctx = contextlib.ExitStack()
        consts = ctx.enter_context(tc.tile_pool(name="consts", bufs=1))
        ph1 = ctx.enter_context(tc.tile_pool(name="ph1", bufs=2))
        ph1o = ctx.enter_context(tc.tile_pool(name="ph1o", bufs=2))
        ph1ps = ctx.enter_context(tc.tile_pool(name="ph1ps", bufs=3, space="PSUM"))
        glo = ctx.enter_context(tc.tile_pool(name="glo", bufs=2))
        ghi = ctx.enter_context(tc.tile_pool(name="ghi", bufs=2))
        stp = ctx.enter_context(tc.tile_pool(name="stp", bufs=2))
        aggps = ctx.enter_context(tc.tile_pool(name="aggps", bufs=4, space="PSUM"))
        misc = ctx.enter_context(tc.tile_pool(name="misc", bufs=1))
        miscps = ctx.enter_context(tc.tile_pool(name="miscps", bufs=1, space="PSUM"))

        # ---- constants ----
        W0 = consts.tile([_P, H], f16, tag="W0")
        W1 = consts.tile([_P, H], f16, tag="W1")
        nc.scalar.dma_start(W0[:], t_W[0:_P, :])
        nc.scalar.dma_start(W1[:], t_W[_P : 2 * _P, :])
        iota_t = consts.tile([_P, _P], f16, tag="iota")
        nc.scalar.dma_start(iota_t[:], t_iota[:])
        b2_sb = consts.tile([1, H2], f32, tag="b2_sb")
        nc.scalar.dma_start(b2_sb[:], t_b2[None, :])
        a_sb = consts.tile([1, 1], f32, tag="a_sb")
        nc.scalar.dma_start(a_sb[:], t_a[None, :])
        dwT0 = consts.tile([_P, H], f32, tag="dwT0")
        dwT1 = consts.tile([_P, H], f32, tag="dwT1")
        nc.scalar.dma_start(dwT0[:], t_dwT[0:_P, :])
        nc.scalar.dma_start(dwT1[:], t_dwT[_P : 2 * _P, :])
        rowmask_sb = consts.tile([_P, 1], f32, tag="rowmask")
        nc.scalar.dma_start(rowmask_sb[:], t_rowmask[:, :])
        ones_row = consts.tile([1, _P], f32, tag="ones_row")
        nc.vector.memset(ones_row[:], 1.0)
        ones_col = consts.tile([_P, 1], f32, tag="ones_col")
        nc.vector.memset(ones_col[:], 1.0)

        ab_ps = miscps.tile([_P, 1], f32, tag="mps")
        nc.tensor.matmul(ab_ps[:], ones_row[:], a_sb[:], start=True, stop=True)
        a_bc = consts.tile([_P, 1], f32, tag="a_bc")
        nc.vector.tensor_copy(a_bc[:], ab_ps[:])

        # ---- stream loads ----
        i1_sb = consts.tile([_P, n_et * 8], mybir.dt.int16, tag="i1")
        w1_sb = consts.tile([_P, n_et], f16, tag="w1")
        d1_sb = consts.tile([_P, n_et], f16, tag="d1")
        nc.scalar.dma_start(i1_sb[:], t_i1[:])
        nc.scalar.dma_start(w1_sb[:], t_w1[:])
        nc.scalar.dma_start(d1_sb[:], t_d1[:])

        # ---- phase 1: xw_cat = [x@W | x_p@W], hi rows first ----
        def phase1_range(r_start, r_end, t_dst):
            for r0 in range(r_start, r_end, CHUNK):
                cols = min(CHUNK, r_end - r0)
                xTb = ph1.tile([_P, 2, CHUNK], f16, tag="xTb")
                xpb = ph1.tile([_P, 2, CHUNK], f16, tag="xpb")
                nc.sync.dma_start(xTb[:, :, :cols], t_xT[:, :, r0 : r0 + cols])
                nc.sync.dma_start(xpb[:, :, :cols], t_xpT[:, :, r0 : r0 + cols])
                # store in 4-group (512-row) batches to cut DMA-issue count
                for s0 in range(0, cols, 4 * _P):
                    srows = min(4 * _P, cols - s0)
                    ng = -(-srows // _P)
                    stage = ph1o.tile([_P, 4, H2], f16, tag="xw_sb")
                    for gi in range(ng):
                        o = s0 + gi * _P
                        m = min(_P, cols - o)
                        ps = ph1ps.tile([_P, H2], f32, tag="ph1ps")
                        nc.tensor.matmul(
                            ps[:m, 0:H], xTb[:, 0, o : o + m], W0[:],
                            start=True, stop=False,
                        )
                        nc.tensor.matmul(
                            ps[:m, 0:H], xTb[:, 1, o : o + m], W1[:],
                            start=False, stop=True,
                        )
                        nc.tensor.matmul(
                            ps[:m, H:H2], xpb[:, 0, o : o + m], W0[:],
                            start=True, stop=False,
                        )
                        nc.tensor.matmul(
                            ps[:m, H:H2], xpb[:, 1, o : o + m], W1[:],
                            start=False, stop=True,
                        )
                        nc.any.tensor_copy(stage[:m, gi, :], ps[:m, :])
                    d0 = r0 + s0 - r_start
                    if srows % _P == 0:
                        nc.sync.dma_start(
                            t_dst[d0 : d0 + srows, :].rearrange(
                                "(g p) h -> p g h", g=ng, p=_P
                            ),
                            stage[:, :ng, :],
                        )
                    else:
                        for gi in range(ng):
                            m = min(_P, srows - gi * _P)
                            nc.sync.dma_start(
                                t_dst[d0 + gi * _P : d0 + gi * _P + m, :],
                                stage[:m, gi, :],
                            )

        phase1_range(_LO, N, t_xw_hi)
        phase1_range(0, _LO, t_xw_lo)

        # ---- aggregation: fused sweep over dst-tile pairs ----
        zcat = consts.tile([_P, DT, H2], f16, tag="zcat")
        cacc = consts.tile([_P, H], f32, tag="cacc")
        nc.vector.memset(cacc[:], 0.0)

        for p in range(NP):
            T_la, T_lb = int(Tm[p, 0, 0]), int(Tm[p, 0, 1])
            T_ha, T_hb = int(Tm[p, 1, 0]), int(Tm[p, 1, 1])
            TL = T_la + T_lb
            TH = T_ha + T_hb
            TT = TL + TH
            o0 = int(Off[p, 0, 0])  # pair stream base; layout lo_a,lo_b,hi_a,hi_b
            gl = gh = None
            if TH:  # hi rows are written first in phase 1 — gather them first
                gh = ghi.tile([_P, maxTH, H2], f16, tag="gh")
                nc.gpsimd.dma_gather(
                    gh[:, :TH, :],
                    t_xw_hi[:, :],
                    i1_sb[:, 8 * (o0 + TL) : 8 * (o0 + TT)],
                    TH * _P,
                    TH * _P,
                    H2,
                    single_packet=(TH * _P <= 1024),
                )
            if TL:
                gl = glo.tile([_P, maxTL, H2], f16, tag="gl")
                nc.gpsimd.dma_gather(
                    gl[:, :TL, :],
                    t_xw_lo[:, :],
                    i1_sb[:, 8 * o0 : 8 * (o0 + TL)],
                    TL * _P,
                    TL * _P,
                    H2,
                    single_packet=(TL * _P <= 1024),
                )
            # batched weighted one-hot for all the pair's edge-tiles
            stw = stp.tile([_P, maxTT, _P], f16, tag="stw")
            nc.vector.tensor_tensor(
                stw[:, :TT, :],
                d1_sb[:, o0 : o0 + TT, None].to_broadcast([_P, TT, _P]),
                iota_t[:, None, :].to_broadcast([_P, TT, _P]),
                mybir.AluOpType.is_equal,
            )
            nc.vector.tensor_tensor(
                stw[:, :TT, :],
                stw[:, :TT, :],
                w1_sb[:, o0 : o0 + TT, None].to_broadcast([_P, TT, _P]),
                mybir.AluOpType.mult,
            )
            for s in range(2):
                dti = 2 * p + s
                if dti >= DT:
                    break
                tl0 = s * T_la          # first lo tile (gl index) of this sub
                tln = T_la if s == 0 else T_lb
                th0 = s * T_ha
                thn = T_ha if s == 0 else T_hb
                ps = aggps.tile([_P, H2], f32, tag="aggps")
                n_mm = tln + thn
                k = 0
                for j in range(tln):
                    nc.tensor.matmul(
                        ps[:],
                        stw[:, tl0 + j, :],
                        gl[:, tl0 + j, :],
                        start=(k == 0),
                        stop=False,
                    )
                    k += 1
                for j in range(thn):
                    nc.tensor.matmul(
                        ps[:],
                        stw[:, TL + th0 + j, :],
                        gh[:, th0 + j, :],
                        start=(k == 0),
                        stop=False,
                    )
                    k += 1
                # bias via K=1 ones x [b|b] matmul (also closes the group)
                nc.tensor.matmul(ps[:], ones_row[:], b2_sb[:], start=(k == 0), stop=True)
                # PReLU in f32 off PSUM
                t1 = misc.tile([_P, H2], f32, tag="t1")
                nc.vector.tensor_scalar(
                    t1[:], ps[:], 0.0, a_bc[:, 0:1],
                    mybir.AluOpType.min, mybir.AluOpType.mult,
                )
                t2 = misc.tile([_P, H2], f32, tag="t2")
                nc.vector.tensor_scalar(t2[:], ps[:], 0.0, None, mybir.AluOpType.max)
                nc.vector.tensor_tensor(t1[:], t1[:], t2[:], mybir.AluOpType.add)
                if dti == DT - 1 and LAST < _P:
                    nc.vector.tensor_scalar(
                        t1[:], t1[:], rowmask_sb[:, 0:1], None, mybir.AluOpType.mult
                    )
                nc.any.tensor_copy(zcat[:, dti, :], t1[:])
                nc.vector.tensor_tensor(
                    cacc[:], cacc[:], t1[:, 0:H], mybir.AluOpType.add
                )

        # ---- summary: column sums of z1 over all nodes ----
        cs_ps = miscps.tile([1, H], f32, tag="mps")
        nc.tensor.matmul(cs_ps[:], ones_col[:], cacc[:], start=True, stop=True)
        cs_sb = misc.tile([1, H], f32, tag="cs_sb")
        nc.vector.tensor_copy(cs_sb[:], cs_ps[:])
        nc.sync.dma_start(t_ar_in[None, :], cs_sb[:])
        nc.gpsimd.collective_compute(
            "AllReduce",
            mybir.AluOpType.add,
            replica_groups=[list(range(C))],
            ins=[t_ar_in[:]],
            outs=[t_ar_out[:]],
        )
        sums_sb = misc.tile([1, H], f32, tag="sums_sb")
        nc.sync.dma_start(sums_sb[:], t_ar_out[None, :])
        summ_sb = misc.tile([1, H], f32, tag="summ_sb")
        nc.scalar.activation(
            summ_sb[:], sums_sb[:], mybir.ActivationFunctionType.Sigmoid,
            scale=1.0 / N,
        )

        # ---- wsum = disc_W @ summary ----
        ident = consts.tile([_P, _P], f32, tag="ident")
        nc.scalar.dma_start(ident[:], t_ident[:])
        sT = misc.tile([_P, 2], f32, tag="sT")
        for c_i in range(2):
            tp = miscps.tile([_P, _P], f32, tag="mps")
            nc.tensor.transpose(
                tp[:, 0:1],
                summ_sb[0:1, c_i * _P : (c_i + 1) * _P],
                ident[0:1, 0:1],
            )
            nc.vector.tensor_copy(sT[:, c_i : c_i + 1], tp[:, 0:1])
        ws_ps = miscps.tile([1, H], f32, tag="mps")
        nc.tensor.matmul(ws_ps[:], sT[:, 0:1], dwT0[:], start=True, stop=False)
        nc.tensor.matmul(ws_ps[:], sT[:, 1:2], dwT1[:], start=False, stop=True)
        ws2_sb = misc.tile([1, H2], f32, tag="ws2_sb")
        nc.vector.tensor_copy(ws2_sb[:, 0:H], ws_ps[:])
        nc.vector.tensor_copy(ws2_sb[:, H:H2], ws_ps[:])
        wb_ps = miscps.tile([_P, H2], f32, tag="mps")
        nc.tensor.matmul(wb_ps[:], ones_row[:], ws2_sb[:], start=True, stop=True)
        wsum_bc = consts.tile([_P, H2], f16, tag="wsum_bc")
        nc.vector.tensor_copy(wsum_bc[:], wb_ps[:])

        # ---- pos/neg dots: zcat *= [wsum|wsum]; reduce 256-chunks ----
        nc.vector.tensor_tensor(
            zcat[:, :, :],
            zcat[:, :, :],
            wsum_bc[:, None, :].to_broadcast([_P, DT, H2]),
            mybir.AluOpType.mult,
        )
        out_acc = misc.tile([_P, DT * 2], f32, tag="out_acc")
        nc.vector.reduce_sum(
            out_acc[:],
            zcat[:, :, :].rearrange("p d (t h) -> p (d t) h", t=2, h=H),
            bass_rust.AxisListType.X,
        )
        nc.sync.dma_start(t_out[:], out_acc[:])
        ctx.close()

    nc.compile()

    in_maps = []
    for c in range(C):
        in_maps.append(
            {
                "xT16": xT_f16,
                "xpT16": xpT_f16,
                "w16": W_f16,
                "b2vec": b2,
                "avec": a,
                "dwT": dwT,
                "iota": iota_np,
                "ident_in": np.eye(_P, dtype=np.float32),
                "rowmask": rowmask_np,
                "idx1": i1[c],
                "wgt1": w1[c],
                "dstl1": d1[c],
            }
        )

    if os.environ.get("KERNEL_SIM", "0") == "1":
        from concourse import bass_interp

        sim = bass_interp.MultiCoreSim(nc, C)
        for c in range(C):
            for k, v in in_maps[c].items():
                sim.cores[c].tensor(k)[:] = v
        sim.simulate()
        results = [
            {"pn_out": np.array(sim.cores[c].tensor("pn_out"))} for c in range(C)
        ]
    else:
        trace = os.environ.get("KERNEL_TRACE", "0") == "1"
        kw = {}
        if trace:
            kw["trace"] = True
        res = run_bass_kernel_spmd(nc, in_maps, core_ids=list(range(C)), **kw)
        kernel.last_result = res
        results = res.results

    pos = np.zeros(N, np.float32)
    neg = np.zeros(N, np.float32)
    for c in range(C):
        arr = results[c]["pn_out"].reshape(_P, DT, 2)
        pos[c * NS : (c + 1) * NS] = arr[:, :, 0].T.reshape(-1)[:NS]
        neg[c * NS : (c + 1) * NS] = arr[:, :, 1].T.reshape(-1)[:NS]
    return pos, neg
